# revision 14
# baseline (speedup 1.0000x reference)
"""RGB->hue + 1x1 conv (scalar scale+bias) Trainium2 Bass kernel.

Problem: x [32,3,512,512] f32 -> out [32,1,512,512] f32
  hue6 selected per argmax(r,g,b) branch:
    r max: (g-b)/delta  (mod 6)
    g max: (b-r)/delta + 2
    b max: (r-g)/delta + 4
  out = hue6 * (W/6) + b

Sharding: pure data parallel, 4 images per core on 8 cores.

Formulation ("centered hue"): let d2 = |g-b|+|b-r|+|r-g| = 2*delta
(sum of pairwise ranges of 3 scalars). Define H = hue6 if hue6<=3 else
hue6-6 (H in (-3,3]). Then
  H*delta = (g-b)        if r is max
          = (b-r) + d2/... (2*delta)  if g is max  -> dbr + d2
          = (r-g) - d2                if b is max  -> drg - d2
  (since +-2*delta == +-d2), and
  hue6-3 = add_range_wrap(H, shift=-3, bound=3, period=6)
replaces the mod-6 wrap with one custom DVE op. Final affine on ACT:
out = w6*(y+3) + bias with w6 = W/6. 1/delta comes from the ACT
Reciprocal spline as 1/(0.5*d2).
"""

import numpy as np

_EXE_CACHE: dict = {}

# Layout constants (hardcoded for x [32,3,512,512] f32, 8 cores)
N_CORES = 8
IMGS_PER_CORE = 4
P = 128              # SBUF partitions
PLANE = 512 * 512    # elements per channel plane
FREE = PLANE // P    # 2048 free-dim elements per plane
FD = 2048            # chunk free-dim size
CHUNKS = FREE // FD  # chunks per image plane


def _build(w6: float, bias: float):
    """Trace the Bass kernel with W/6 and bias baked as immediates."""
    import concourse.bacc as bacc
    import concourse.bass as bass
    import concourse.tile as tile
    from concourse import mybir

    F32 = mybir.dt.float32
    BF16 = mybir.dt.bfloat16
    U16 = mybir.dt.uint16
    Alu = mybir.AluOpType
    Act = mybir.ActivationFunctionType
    ts = bass.ts

    nc = bacc.Bacc("TRN2", target_bir_lowering=False, debug=False)

    def act_recip(out_ap, in_ap, scale=1.0):
        # Direct InstActivation emission: the bass wrapper refuses
        # Reciprocal for accuracy reasons; ~1e-4 rel here is plenty.
        ins = [
            nc.scalar.lower_ap(in_ap),
            mybir.ImmediateValue(dtype=F32, value=0.0),       # bias
            mybir.ImmediateValue(dtype=F32, value=scale),     # scale
            mybir.ImmediateValue(dtype=F32, value=0.0),       # alpha
        ]
        return nc.scalar.add_instruction(
            mybir.InstActivation(
                name=nc.get_next_instruction_name(),
                func=Act.Reciprocal,
                ins=ins,
                outs=[nc.scalar.lower_ap(out_ap)],
            )
        )

    x_t = nc.dram_tensor("x", [IMGS_PER_CORE * 3, P, FREE], F32, kind="ExternalInput")
    o_t = nc.dram_tensor("out", [IMGS_PER_CORE, P, FREE], F32, kind="ExternalOutput")

    NCHUNK = IMGS_PER_CORE * CHUNKS

    with tile.TileContext(nc, pool_alloc_mode="queue") as tc:
        with (
            tc.tile_pool(name="io", bufs=2) as io,
            tc.tile_pool(name="tmp", bufs=2) as tmp,
        ):
            state = {}

            def stage_a(ci):
                img, h = divmod(ci, CHUNKS)
                r = io.tile([P, FD], F32, tag="r")
                g = io.tile([P, FD], F32, tag="g")
                b = io.tile([P, FD], F32, tag="b")
                nc.sync.dma_start(r[:], x_t[img * 3 + 0, :, ts(h, FD)])
                nc.sync.dma_start(g[:], x_t[img * 3 + 1, :, ts(h, FD)])
                nc.sync.dma_start(b[:], x_t[img * 3 + 2, :, ts(h, FD)])

                dgb = tmp.tile([P, FD], BF16, tag="dgb")
                dbr = tmp.tile([P, FD], BF16, tag="dbr")
                drg = tmp.tile([P, FD], BF16, tag="drg")
                nc.gpsimd.tensor_sub(dgb[:], g[:], b[:])
                nc.gpsimd.tensor_sub(dbr[:], b[:], r[:])
                nc.gpsimd.tensor_sub(drg[:], r[:], g[:])

                # Branch masks (u16) — only need the diffs, issue early:
                #   s1 = (dgb>0)            -> select g-branch
                #   s3 = (drg>=0)           -> half of r-branch mask
                s1 = tmp.tile([P, FD], U16, tag="s1")
                s3 = tmp.tile([P, FD], U16, tag="s3")
                nc.vector.tensor_scalar(
                    out=s1[:], in0=dgb[:], scalar1=0.0, scalar2=None,
                    op0=Alu.is_gt,
                )
                nc.vector.tensor_scalar(
                    out=s3[:], in0=drg[:], scalar1=0.0, scalar2=None,
                    op0=Alu.is_ge,
                )

                # d2 = |dgb|+|dbr|+|drg| = 2*delta  (abs on ACT)
                a1 = tmp.tile([P, FD], BF16, tag="a1")
                a2 = tmp.tile([P, FD], BF16, tag="a2")
                a3 = tmp.tile([P, FD], BF16, tag="a3")
                nc.scalar.activation(a1[:], dgb[:], Act.Abs)
                nc.scalar.activation(a2[:], dbr[:], Act.Abs)
                nc.scalar.activation(a3[:], drg[:], Act.Abs)
                d2 = tmp.tile([P, FD], BF16, tag="d2")
                nc.vector.tensor_add(d2[:], a1[:], a2[:])
                nc.vector.tensor_add(d2[:], d2[:], a3[:])

                # u = 1/delta = Recip(0.5*d2) on ACT
                u = tmp.tile([P, FD], BF16, tag="u")
                act_recip(u[:], d2[:], scale=0.5)

                state[ci] = (dgb, dbr, drg, d2, u, s1, s3)

            def stage_b(ci):
                img, h = divmod(ci, CHUNKS)
                dgb, dbr, drg, d2, u, s1, s3 = state.pop(ci)

                # r-branch mask: c1 = (drg>=0)&(dbr<=0)
                c1 = tmp.tile([P, FD], U16, tag="c1")
                nc.vector.scalar_tensor_tensor(
                    c1[:], dbr[:], 0.0, s3[:], op0=Alu.is_le,
                    op1=Alu.logical_and,
                )

                # Branch candidates (H*delta):
                #   b-max: drg - d2 (default), g-max: dbr + d2, r-max: dgb
                cb = tmp.tile([P, FD], BF16, tag="cb")
                cg = tmp.tile([P, FD], BF16, tag="cg")
                nc.vector.tensor_sub(cb[:], drg[:], d2[:])
                nc.vector.tensor_add(cg[:], dbr[:], d2[:])
                nc.vector.copy_predicated(cb[:], s1[:], cg[:])
                nc.vector.copy_predicated(cb[:], c1[:], dgb[:])

                # y = H = (H*delta)*(1/delta); wrap:
                # y2 = (y-3) + 6*[(y-3) < -3] = hue6 - 3
                nc.vector.tensor_tensor(cb[:], cb[:], u[:], op=Alu.mult)
                y2 = tmp.tile([P, FD], BF16, tag="y2")
                nc.vector.add_range_wrap(y2[:], cb[:], -3.0, 3.0, 6.0)

                # out = w6*(y2+3) + bias on ACT
                o = io.tile([P, FD], F32, tag="o")
                nc.scalar.activation(
                    o[:], y2[:], Act.Copy, bias=bias + 3.0 * w6, scale=w6
                )

                # output DMA from the (idle) PE engine's queue so it never
                # head-of-line blocks the input DMAs on sync
                nc.scalar.dma_start(o_t[img, :, ts(h, FD)], o[:])

            # software pipeline, skew 1: A(0) A(1) B(0) A(2) B(1) ...
            for ci in range(NCHUNK + 1):
                if ci < NCHUNK:
                    stage_a(ci)
                if ci >= 1:
                    stage_b(ci - 1)

    nc.compile()
    return nc


def _get_nc(w6: float, bias: float):
    key = (w6, bias, FD)
    if key not in _EXE_CACHE:
        _EXE_CACHE[key] = _build(w6, bias)
    return _EXE_CACHE[key]


def _run(x, W, b, trace=False, tmpdir=None):
    from concourse.bass_utils import run_bass_kernel_spmd

    x = np.ascontiguousarray(np.asarray(x, dtype=np.float32))
    Wv = float(np.asarray(W).reshape(-1)[0])
    bv = float(np.asarray(b).reshape(-1)[0])
    w6 = Wv / 6.0

    nc = _get_nc(w6, bv)

    shards = x.reshape(N_CORES, IMGS_PER_CORE * 3, P, FREE)
    in_maps = [{"x": shards[i]} for i in range(N_CORES)]
    res = run_bass_kernel_spmd(
        nc, in_maps, list(range(N_CORES)), trace=trace, tmpdir=tmpdir
    )
    out = np.stack([res.results[i]["out"] for i in range(N_CORES)], axis=0)
    out = out.reshape(32, 1, 512, 512)
    return out, res


def kernel(x, W, b):
    out, _ = _run(x, W, b, trace=False)
    return out


# revision 15
# speedup vs baseline: 1.1988x; 1.1988x over previous
"""RGB->hue + 1x1 conv (scalar scale+bias) Trainium2 Bass kernel.

Problem: x [32,3,512,512] f32 -> out [32,1,512,512] f32
  hue6 selected per argmax(r,g,b) branch:
    r max: (g-b)/delta  (mod 6)
    g max: (b-r)/delta + 2
    b max: (r-g)/delta + 4
  out = hue6 * (W/6) + b

Sharding: pure data parallel, 4 images per core on 8 cores.

Formulation ("centered hue"): let d2 = |g-b|+|b-r|+|r-g| = 2*delta
(sum of pairwise ranges of 3 scalars). Define H = hue6 if hue6<=3 else
hue6-6 (H in (-3,3]). Then
  H*delta = (g-b)        if r is max
          = (b-r) + d2/... (2*delta)  if g is max  -> dbr + d2
          = (r-g) - d2                if b is max  -> drg - d2
  (since +-2*delta == +-d2), and
  hue6-3 = add_range_wrap(H, shift=-3, bound=3, period=6)
replaces the mod-6 wrap with one custom DVE op. Final affine on ACT:
out = w6*(y+3) + bias with w6 = W/6. 1/delta comes from the ACT
Reciprocal spline as 1/(0.5*d2).
"""

import numpy as np

_EXE_CACHE: dict = {}

# Layout constants (hardcoded for x [32,3,512,512] f32, 8 cores)
N_CORES = 8
IMGS_PER_CORE = 4
P = 128              # SBUF partitions
PLANE = 512 * 512    # elements per channel plane
FREE = PLANE // P    # 2048 free-dim elements per plane
FD = 2048            # chunk free-dim size
CHUNKS = FREE // FD  # chunks per image plane


def _build(w6: float, bias: float):
    """Trace the Bass kernel with W/6 and bias baked as immediates."""
    import concourse.bacc as bacc
    import concourse.bass as bass
    import concourse.tile as tile
    from concourse import mybir

    F32 = mybir.dt.float32
    BF16 = mybir.dt.bfloat16
    U16 = mybir.dt.uint16
    Alu = mybir.AluOpType
    Act = mybir.ActivationFunctionType
    ts = bass.ts

    nc = bacc.Bacc("TRN2", target_bir_lowering=False, debug=False)

    def act_recip(out_ap, in_ap, scale=1.0):
        # Direct InstActivation emission: the bass wrapper refuses
        # Reciprocal for accuracy reasons; ~1e-4 rel here is plenty.
        ins = [
            nc.scalar.lower_ap(in_ap),
            mybir.ImmediateValue(dtype=F32, value=0.0),       # bias
            mybir.ImmediateValue(dtype=F32, value=scale),     # scale
            mybir.ImmediateValue(dtype=F32, value=0.0),       # alpha
        ]
        return nc.scalar.add_instruction(
            mybir.InstActivation(
                name=nc.get_next_instruction_name(),
                func=Act.Reciprocal,
                ins=ins,
                outs=[nc.scalar.lower_ap(out_ap)],
            )
        )

    x_t = nc.dram_tensor("x", [IMGS_PER_CORE * 3, P, FREE], F32, kind="ExternalInput")
    o_t = nc.dram_tensor("out", [IMGS_PER_CORE, P, FREE], F32, kind="ExternalOutput")

    NCHUNK = IMGS_PER_CORE * CHUNKS

    with tile.TileContext(nc, pool_alloc_mode="queue") as tc:
        with (
            tc.tile_pool(name="io", bufs=2) as io,
            tc.tile_pool(name="tmp", bufs=2) as tmp,
        ):
            state = {}

            def stage_a(ci):
                img, h = divmod(ci, CHUNKS)
                r = io.tile([P, FD], F32, tag="r")
                g = io.tile([P, FD], F32, tag="g")
                b = io.tile([P, FD], F32, tag="b")
                nc.sync.dma_start(r[:], x_t[img * 3 + 0, :, ts(h, FD)])
                nc.sync.dma_start(g[:], x_t[img * 3 + 1, :, ts(h, FD)])
                nc.sync.dma_start(b[:], x_t[img * 3 + 2, :, ts(h, FD)])

                # All elementwise on Vector/ACT only: GpSimd shares its SBUF
                # port with the DVE and slows concurrent Vector ops ~4x.
                # drg is never materialized: ndrg = dgb+dbr = -(r-g) is a
                # cheap bf16 add, and the b-branch candidate uses 2*d2-ndrg
                # (the +6*delta hue shift is absorbed by the range wrap).
                dgb = tmp.tile([P, FD], BF16, tag="dgb")
                dbr = tmp.tile([P, FD], BF16, tag="dbr")
                ndrg = tmp.tile([P, FD], BF16, tag="ndrg")
                nc.vector.tensor_sub(dgb[:], g[:], b[:])
                nc.vector.tensor_sub(dbr[:], b[:], r[:])
                nc.vector.tensor_add(ndrg[:], dgb[:], dbr[:])

                # Branch masks (u16) — only need the diffs, issue early:
                #   s1 = (dgb>0)             -> select g-branch
                #   s3 = (ndrg<=0) = (drg>=0) -> half of r-branch mask
                s1 = tmp.tile([P, FD], U16, tag="s1")
                s3 = tmp.tile([P, FD], U16, tag="s3")
                nc.vector.tensor_scalar(
                    out=s1[:], in0=dgb[:], scalar1=0.0, scalar2=None,
                    op0=Alu.is_gt,
                )
                nc.vector.tensor_scalar(
                    out=s3[:], in0=ndrg[:], scalar1=0.0, scalar2=None,
                    op0=Alu.is_le,
                )

                # d2 = |dgb|+|dbr|+|drg| = 2*delta  (abs on ACT)
                a1 = tmp.tile([P, FD], BF16, tag="a1")
                a2 = tmp.tile([P, FD], BF16, tag="a2")
                a3 = tmp.tile([P, FD], BF16, tag="a3")
                nc.scalar.activation(a1[:], dgb[:], Act.Abs)
                nc.scalar.activation(a2[:], dbr[:], Act.Abs)
                nc.scalar.activation(a3[:], ndrg[:], Act.Abs)

                state[ci] = (dgb, dbr, ndrg, a1, a2, a3, s1, s3)

            def stage_b(ci):
                img, h = divmod(ci, CHUNKS)
                dgb, dbr, ndrg, a1, a2, a3, s1, s3 = state.pop(ci)

                d2 = tmp.tile([P, FD], BF16, tag="d2")
                nc.vector.tensor_add(d2[:], a1[:], a2[:])
                nc.vector.tensor_add(d2[:], d2[:], a3[:])

                # u = 1/delta = Recip(0.5*d2) on ACT
                u = tmp.tile([P, FD], BF16, tag="u")
                act_recip(u[:], d2[:], scale=0.5)

                # r-branch mask: c1 = (drg>=0)&(dbr<=0)
                c1 = tmp.tile([P, FD], U16, tag="c1")
                nc.vector.scalar_tensor_tensor(
                    c1[:], dbr[:], 0.0, s3[:], op0=Alu.is_le,
                    op1=Alu.logical_and,
                )

                # Branch candidates (H*delta, b-branch shifted +6*delta):
                #   b-max: 2*d2-ndrg (default), g-max: dbr + d2, r-max: dgb
                cb = tmp.tile([P, FD], BF16, tag="cb")
                cg = tmp.tile([P, FD], BF16, tag="cg")
                nc.vector.scalar_tensor_tensor(
                    cb[:], d2[:], 2.0, ndrg[:], op0=Alu.mult, op1=Alu.subtract
                )
                nc.vector.tensor_add(cg[:], dbr[:], d2[:])
                nc.vector.copy_predicated(cb[:], s1[:], cg[:])
                nc.vector.copy_predicated(cb[:], c1[:], dgb[:])

                # y = H = (H*delta)*(1/delta) in [-1,5]; wrap:
                # y2 = (y-3) + 6*[(y-3) < -3] = hue6 - 3
                nc.vector.tensor_tensor(cb[:], cb[:], u[:], op=Alu.mult)
                y2 = tmp.tile([P, FD], BF16, tag="y2")
                nc.vector.add_range_wrap(y2[:], cb[:], -3.0, 3.0, 6.0)

                # out = w6*(y2+3) + bias on ACT
                o = io.tile([P, FD], F32, tag="o")
                nc.scalar.activation(
                    o[:], y2[:], Act.Copy, bias=bias + 3.0 * w6, scale=w6
                )

                # output DMA from the (idle) PE engine's queue so it never
                # head-of-line blocks the input DMAs on sync
                nc.scalar.dma_start(o_t[img, :, ts(h, FD)], o[:])

            # software pipeline, skew 1: A(0) A(1) B(0) A(2) B(1) ...
            for ci in range(NCHUNK + 1):
                if ci < NCHUNK:
                    stage_a(ci)
                if ci >= 1:
                    stage_b(ci - 1)

    nc.compile()
    return nc


def _get_nc(w6: float, bias: float):
    key = (w6, bias, FD)
    if key not in _EXE_CACHE:
        _EXE_CACHE[key] = _build(w6, bias)
    return _EXE_CACHE[key]


def _run(x, W, b, trace=False, tmpdir=None):
    from concourse.bass_utils import run_bass_kernel_spmd

    x = np.ascontiguousarray(np.asarray(x, dtype=np.float32))
    Wv = float(np.asarray(W).reshape(-1)[0])
    bv = float(np.asarray(b).reshape(-1)[0])
    w6 = Wv / 6.0

    nc = _get_nc(w6, bv)

    shards = x.reshape(N_CORES, IMGS_PER_CORE * 3, P, FREE)
    in_maps = [{"x": shards[i]} for i in range(N_CORES)]
    res = run_bass_kernel_spmd(
        nc, in_maps, list(range(N_CORES)), trace=trace, tmpdir=tmpdir
    )
    out = np.stack([res.results[i]["out"] for i in range(N_CORES)], axis=0)
    out = out.reshape(32, 1, 512, 512)
    return out, res


def kernel(x, W, b):
    out, _ = _run(x, W, b, trace=False)
    return out


# revision 18
# speedup vs baseline: 1.2191x; 1.0169x over previous
"""RGB->hue + 1x1 conv (scalar scale+bias) Trainium2 Bass kernel.

Problem: x [32,3,512,512] f32 -> out [32,1,512,512] f32
  hue6 selected per argmax(r,g,b) branch:
    r max: (g-b)/delta  (mod 6)
    g max: (b-r)/delta + 2
    b max: (r-g)/delta + 4
  out = hue6 * (W/6) + b

Sharding: pure data parallel, 4 images per core on 8 cores.

Formulation ("centered hue"): let d2 = |g-b|+|b-r|+|r-g| = 2*delta
(sum of pairwise ranges of 3 scalars). Define H = hue6 if hue6<=3 else
hue6-6 (H in (-3,3]). Then
  H*delta = (g-b)        if r is max
          = (b-r) + d2/... (2*delta)  if g is max  -> dbr + d2
          = (r-g) - d2                if b is max  -> drg - d2
  (since +-2*delta == +-d2), and
  hue6-3 = add_range_wrap(H, shift=-3, bound=3, period=6)
replaces the mod-6 wrap with one custom DVE op. Final affine on ACT:
out = w6*(y+3) + bias with w6 = W/6. 1/delta comes from the ACT
Reciprocal spline as 1/(0.5*d2).
"""

import numpy as np

_EXE_CACHE: dict = {}

# Layout constants (hardcoded for x [32,3,512,512] f32, 8 cores)
N_CORES = 8
IMGS_PER_CORE = 4
P = 128              # SBUF partitions
PLANE = 512 * 512    # elements per channel plane
FREE = PLANE // P    # 2048 free-dim elements per plane
FD = 2048            # chunk free-dim size
CHUNKS = FREE // FD  # chunks per image plane


def _build(w6: float, bias: float):
    """Trace the Bass kernel with W/6 and bias baked as immediates."""
    import concourse.bacc as bacc
    import concourse.bass as bass
    import concourse.tile as tile
    from concourse import mybir

    F32 = mybir.dt.float32
    BF16 = mybir.dt.bfloat16
    U8 = mybir.dt.uint8
    Alu = mybir.AluOpType
    Act = mybir.ActivationFunctionType
    ts = bass.ts

    nc = bacc.Bacc("TRN2", target_bir_lowering=False, debug=False)

    def act_recip(out_ap, in_ap, scale=1.0):
        # Direct InstActivation emission: the bass wrapper refuses
        # Reciprocal for accuracy reasons; ~1e-4 rel here is plenty.
        ins = [
            nc.scalar.lower_ap(in_ap),
            mybir.ImmediateValue(dtype=F32, value=0.0),       # bias
            mybir.ImmediateValue(dtype=F32, value=scale),     # scale
            mybir.ImmediateValue(dtype=F32, value=0.0),       # alpha
        ]
        return nc.scalar.add_instruction(
            mybir.InstActivation(
                name=nc.get_next_instruction_name(),
                func=Act.Reciprocal,
                ins=ins,
                outs=[nc.scalar.lower_ap(out_ap)],
            )
        )

    x_t = nc.dram_tensor("x", [IMGS_PER_CORE * 3, P, FREE], F32, kind="ExternalInput")
    o_t = nc.dram_tensor("out", [IMGS_PER_CORE, P, FREE], F32, kind="ExternalOutput")

    NCHUNK = IMGS_PER_CORE * CHUNKS

    with tile.TileContext(nc, pool_alloc_mode="queue") as tc:
        with (
            tc.tile_pool(name="io", bufs=2) as io,
            tc.tile_pool(name="tmp", bufs=2) as tmp,
        ):
            state = {}

            def stage_a(ci):
                img, h = divmod(ci, CHUNKS)
                r = io.tile([P, FD], F32, tag="r")
                g = io.tile([P, FD], F32, tag="g")
                b = io.tile([P, FD], F32, tag="b")
                nc.sync.dma_start(r[:], x_t[img * 3 + 0, :, ts(h, FD)])
                nc.sync.dma_start(g[:], x_t[img * 3 + 1, :, ts(h, FD)])
                nc.sync.dma_start(b[:], x_t[img * 3 + 2, :, ts(h, FD)])

                # All elementwise on Vector/ACT only: GpSimd shares its SBUF
                # port with the DVE and slows concurrent Vector ops ~4x.
                # drg is never materialized: ndrg = dgb+dbr = -(r-g) is a
                # cheap bf16 add, and the b-branch candidate uses 2*d2-ndrg
                # (the +6*delta hue shift is absorbed by the range wrap).
                dgb = tmp.tile([P, FD], BF16, tag="dgb")
                dbr = tmp.tile([P, FD], BF16, tag="dbr")
                ndrg = tmp.tile([P, FD], BF16, tag="ndrg")
                nc.vector.tensor_sub(dgb[:], g[:], b[:])
                nc.vector.tensor_sub(dbr[:], b[:], r[:])
                nc.vector.tensor_add(ndrg[:], dgb[:], dbr[:])

                # Branch masks on ACT via Sign (u8 cast saturates -1 to 0):
                #   s1 = (dgb>0)   -> select g-branch
                #   s2 = (dbr<0), s3 = (drg>0): r-branch = s2 & s3
                s1 = tmp.tile([P, FD], U8, tag="s1")
                s2 = tmp.tile([P, FD], U8, tag="s2")
                s3 = tmp.tile([P, FD], U8, tag="s3")
                nc.scalar.activation(s1[:], dgb[:], Act.Sign)
                nc.scalar.activation(s2[:], dbr[:], Act.Sign, scale=-1.0)
                nc.scalar.activation(s3[:], ndrg[:], Act.Sign, scale=-1.0)

                # d2 = |dgb|+|dbr|+|drg| = 2*delta  (abs on ACT)
                a1 = tmp.tile([P, FD], BF16, tag="a1")
                a2 = tmp.tile([P, FD], BF16, tag="a2")
                a3 = tmp.tile([P, FD], BF16, tag="a3")
                nc.scalar.activation(a1[:], dgb[:], Act.Abs)
                nc.scalar.activation(a2[:], dbr[:], Act.Abs)
                nc.scalar.activation(a3[:], ndrg[:], Act.Abs)

                state[ci] = (dgb, dbr, ndrg, a1, a2, a3, s1, s2, s3)

            def stage_b(ci):
                img, h = divmod(ci, CHUNKS)
                dgb, dbr, ndrg, a1, a2, a3, s1, s2, s3 = state.pop(ci)

                d2 = tmp.tile([P, FD], BF16, tag="d2")
                nc.vector.tensor_add(d2[:], a1[:], a2[:])
                nc.vector.tensor_add(d2[:], d2[:], a3[:])

                # u = 1/delta = Recip(0.5*d2) on ACT
                u = tmp.tile([P, FD], BF16, tag="u")
                act_recip(u[:], d2[:], scale=0.5)

                # r-branch mask: c1 = (drg>0)&(dbr<0)
                c1 = tmp.tile([P, FD], U8, tag="c1")
                nc.vector.tensor_tensor(c1[:], s3[:], s2[:], op=Alu.logical_and)

                # Branch candidates (H*delta, b-branch shifted +6*delta):
                #   b-max: 2*d2-ndrg (default), g-max: dbr + d2, r-max: dgb
                cb = tmp.tile([P, FD], BF16, tag="cb")
                cg = tmp.tile([P, FD], BF16, tag="cg")
                nc.vector.scalar_tensor_tensor(
                    cb[:], d2[:], 2.0, ndrg[:], op0=Alu.mult, op1=Alu.subtract
                )
                nc.vector.tensor_add(cg[:], dbr[:], d2[:])
                nc.vector.copy_predicated(cb[:], s1[:], cg[:])
                nc.vector.copy_predicated(cb[:], c1[:], dgb[:])

                # y = H = (H*delta)*(1/delta) in [-1,5]; wrap:
                # y2 = (y-3) + 6*[(y-3) < -3] = hue6 - 3
                nc.vector.tensor_tensor(cb[:], cb[:], u[:], op=Alu.mult)
                y2 = tmp.tile([P, FD], BF16, tag="y2")
                nc.vector.add_range_wrap(y2[:], cb[:], -3.0, 3.0, 6.0)

                # out = w6*(y2+3) + bias on ACT
                o = io.tile([P, FD], F32, tag="o")
                nc.scalar.activation(
                    o[:], y2[:], Act.Copy, bias=bias + 3.0 * w6, scale=w6
                )

                # output DMA from the (idle) PE engine's queue so it never
                # head-of-line blocks the input DMAs on sync
                nc.scalar.dma_start(o_t[img, :, ts(h, FD)], o[:])

            # software pipeline, skew 1: A(0) A(1) B(0) A(2) B(1) ...
            for ci in range(NCHUNK + 1):
                if ci < NCHUNK:
                    stage_a(ci)
                if ci >= 1:
                    stage_b(ci - 1)

    nc.compile()
    return nc


def _get_nc(w6: float, bias: float):
    key = (w6, bias, FD)
    if key not in _EXE_CACHE:
        _EXE_CACHE[key] = _build(w6, bias)
    return _EXE_CACHE[key]


def _run(x, W, b, trace=False, tmpdir=None):
    from concourse.bass_utils import run_bass_kernel_spmd

    x = np.ascontiguousarray(np.asarray(x, dtype=np.float32))
    Wv = float(np.asarray(W).reshape(-1)[0])
    bv = float(np.asarray(b).reshape(-1)[0])
    w6 = Wv / 6.0

    nc = _get_nc(w6, bv)

    shards = x.reshape(N_CORES, IMGS_PER_CORE * 3, P, FREE)
    in_maps = [{"x": shards[i]} for i in range(N_CORES)]
    res = run_bass_kernel_spmd(
        nc, in_maps, list(range(N_CORES)), trace=trace, tmpdir=tmpdir
    )
    out = np.stack([res.results[i]["out"] for i in range(N_CORES)], axis=0)
    out = out.reshape(32, 1, 512, 512)
    return out, res


def kernel(x, W, b):
    out, _ = _run(x, W, b, trace=False)
    return out


# revision 19
# speedup vs baseline: 1.2733x; 1.0444x over previous
"""RGB->hue + 1x1 conv (scalar scale+bias) Trainium2 Bass kernel.

Problem: x [32,3,512,512] f32 -> out [32,1,512,512] f32
  hue6 selected per argmax(r,g,b) branch:
    r max: (g-b)/delta  (mod 6)
    g max: (b-r)/delta + 2
    b max: (r-g)/delta + 4
  out = hue6 * (W/6) + b

Sharding: pure data parallel, 4 images per core on 8 cores.

Formulation ("centered hue"): let d2 = |g-b|+|b-r|+|r-g| = 2*delta
(sum of pairwise ranges of 3 scalars). Define H = hue6 if hue6<=3 else
hue6-6 (H in (-3,3]). Then
  H*delta = (g-b)        if r is max
          = (b-r) + d2/... (2*delta)  if g is max  -> dbr + d2
          = (r-g) - d2                if b is max  -> drg - d2
  (since +-2*delta == +-d2), and
  hue6-3 = add_range_wrap(H, shift=-3, bound=3, period=6)
replaces the mod-6 wrap with one custom DVE op. Final affine on ACT:
out = w6*(y+3) + bias with w6 = W/6. 1/delta comes from the ACT
Reciprocal spline as 1/(0.5*d2).
"""

import numpy as np

_EXE_CACHE: dict = {}

# Layout constants (hardcoded for x [32,3,512,512] f32, 8 cores)
N_CORES = 8
IMGS_PER_CORE = 4
P = 128              # SBUF partitions
PLANE = 512 * 512    # elements per channel plane
FREE = PLANE // P    # 2048 free-dim elements per plane
FD = 2048            # chunk free-dim size
CHUNKS = FREE // FD  # chunks per image plane


def _build(w6: float, bias: float):
    """Trace the Bass kernel with W/6 and bias baked as immediates."""
    import concourse.bacc as bacc
    import concourse.bass as bass
    import concourse.tile as tile
    from concourse import mybir

    F32 = mybir.dt.float32
    BF16 = mybir.dt.bfloat16
    U8 = mybir.dt.uint8
    Alu = mybir.AluOpType
    Act = mybir.ActivationFunctionType
    ts = bass.ts

    nc = bacc.Bacc("TRN2", target_bir_lowering=False, debug=False)

    def act_recip(out_ap, in_ap, scale=1.0):
        # Direct InstActivation emission: the bass wrapper refuses
        # Reciprocal for accuracy reasons; ~1e-4 rel here is plenty.
        ins = [
            nc.scalar.lower_ap(in_ap),
            mybir.ImmediateValue(dtype=F32, value=0.0),       # bias
            mybir.ImmediateValue(dtype=F32, value=scale),     # scale
            mybir.ImmediateValue(dtype=F32, value=0.0),       # alpha
        ]
        return nc.scalar.add_instruction(
            mybir.InstActivation(
                name=nc.get_next_instruction_name(),
                func=Act.Reciprocal,
                ins=ins,
                outs=[nc.scalar.lower_ap(out_ap)],
            )
        )

    x_t = nc.dram_tensor("x", [IMGS_PER_CORE * 3, P, FREE], F32, kind="ExternalInput")
    o_t = nc.dram_tensor("out", [IMGS_PER_CORE, P, FREE], F32, kind="ExternalOutput")

    NCHUNK = IMGS_PER_CORE * CHUNKS

    with tile.TileContext(nc, pool_alloc_mode="queue") as tc:
        with (
            tc.tile_pool(name="io", bufs=2) as io,
            tc.tile_pool(name="tmp", bufs=2) as tmp,
        ):
            state = {}

            def stage_a(ci):
                img, h = divmod(ci, CHUNKS)
                r = io.tile([P, FD], F32, tag="r")
                g = io.tile([P, FD], F32, tag="g")
                b = io.tile([P, FD], F32, tag="b")
                nc.sync.dma_start(r[:], x_t[img * 3 + 0, :, ts(h, FD)])
                nc.sync.dma_start(g[:], x_t[img * 3 + 1, :, ts(h, FD)])
                nc.sync.dma_start(b[:], x_t[img * 3 + 2, :, ts(h, FD)])

                # All elementwise on Vector/ACT only: GpSimd shares its SBUF
                # port with the DVE and slows concurrent Vector ops ~4x.
                # drg is never materialized: ndrg = dgb+dbr = -(r-g) is a
                # cheap bf16 add, and the b-branch candidate uses 2*d2-ndrg
                # (the +6*delta hue shift is absorbed by the range wrap).
                dgb = tmp.tile([P, FD], BF16, tag="dgb")
                dbr = tmp.tile([P, FD], BF16, tag="dbr")
                ndrg = tmp.tile([P, FD], BF16, tag="ndrg")
                nc.vector.tensor_sub(dgb[:], g[:], b[:])
                nc.vector.tensor_sub(dbr[:], b[:], r[:])
                nc.vector.tensor_add(ndrg[:], dgb[:], dbr[:])

                # Branch masks on ACT:
                #   s1 = (dgb>0) u8 (Sign; u8 cast saturates -1 to 0)
                #   s2f = BIG*relu(-dbr), s3f = BIG*relu(-ndrg) as bf16;
                #   r-branch mask = (s2f*s3f != 0) via one bf16 multiply
                s1 = tmp.tile([P, FD], U8, tag="s1")
                s2 = tmp.tile([P, FD], BF16, tag="s2")
                s3 = tmp.tile([P, FD], BF16, tag="s3")
                nc.scalar.activation(s1[:], dgb[:], Act.Sign)
                nc.scalar.activation(s2[:], dbr[:], Act.Relu, scale=-1e4)
                nc.scalar.activation(s3[:], ndrg[:], Act.Relu, scale=-1e4)

                # d4 = 2*(|dgb|+|dbr|+|drg|) = 4*delta (abs pre-doubled, free)
                a1 = tmp.tile([P, FD], BF16, tag="a1")
                a2 = tmp.tile([P, FD], BF16, tag="a2")
                a3 = tmp.tile([P, FD], BF16, tag="a3")
                nc.scalar.activation(a1[:], dgb[:], Act.Abs, scale=2.0)
                nc.scalar.activation(a2[:], dbr[:], Act.Abs, scale=2.0)
                nc.scalar.activation(a3[:], ndrg[:], Act.Abs, scale=2.0)

                state[ci] = (dgb, dbr, ndrg, a1, a2, a3, s1, s2, s3)

            def stage_b(ci):
                img, h = divmod(ci, CHUNKS)
                dgb, dbr, ndrg, a1, a2, a3, s1, s2, s3 = state.pop(ci)

                d4 = tmp.tile([P, FD], BF16, tag="d4")
                nc.vector.tensor_add(d4[:], a1[:], a2[:])
                nc.vector.tensor_add(d4[:], d4[:], a3[:])

                # u = 1/delta = Recip(0.25*d4) on ACT
                u = tmp.tile([P, FD], BF16, tag="u")
                act_recip(u[:], d4[:], scale=0.25)

                # r-branch mask: c1 = (drg>0)&(dbr<0) as bf16 product -> u8
                c1 = tmp.tile([P, FD], U8, tag="c1")
                nc.vector.tensor_tensor(c1[:], s3[:], s2[:], op=Alu.mult)

                # Branch candidates (H*delta; g shifted -6d, b shifted +6d —
                # both absorbed by the wrap):
                #   b-max: drg+4d = d4-ndrg (default)
                #   g-max: dbr-4d = dbr-d4, r-max: dgb
                cb = tmp.tile([P, FD], BF16, tag="cb")
                cg = tmp.tile([P, FD], BF16, tag="cg")
                nc.vector.tensor_sub(cb[:], d4[:], ndrg[:])
                nc.vector.tensor_sub(cg[:], dbr[:], d4[:])
                nc.vector.copy_predicated(cb[:], s1[:], cg[:])
                nc.vector.copy_predicated(cb[:], c1[:], dgb[:])

                # y = (H*delta)*(1/delta) in [-5,5]; wrap adds 6 iff y<0:
                # y2 = (y-3) + 6*[(y-3) < -3] = hue6 - 3
                nc.vector.tensor_tensor(cb[:], cb[:], u[:], op=Alu.mult)
                y2 = tmp.tile([P, FD], BF16, tag="y2")
                nc.vector.add_range_wrap(y2[:], cb[:], -3.0, 3.0, 6.0)

                # out = w6*(y2+3) + bias on ACT
                o = io.tile([P, FD], F32, tag="o")
                nc.scalar.activation(
                    o[:], y2[:], Act.Copy, bias=bias + 3.0 * w6, scale=w6
                )

                # output DMA from the (idle) PE engine's queue so it never
                # head-of-line blocks the input DMAs on sync
                nc.scalar.dma_start(o_t[img, :, ts(h, FD)], o[:])

            # software pipeline, skew 1: A(0) A(1) B(0) A(2) B(1) ...
            for ci in range(NCHUNK + 1):
                if ci < NCHUNK:
                    stage_a(ci)
                if ci >= 1:
                    stage_b(ci - 1)

    nc.compile()
    return nc


def _get_nc(w6: float, bias: float):
    key = (w6, bias, FD)
    if key not in _EXE_CACHE:
        _EXE_CACHE[key] = _build(w6, bias)
    return _EXE_CACHE[key]


def _run(x, W, b, trace=False, tmpdir=None):
    from concourse.bass_utils import run_bass_kernel_spmd

    x = np.ascontiguousarray(np.asarray(x, dtype=np.float32))
    Wv = float(np.asarray(W).reshape(-1)[0])
    bv = float(np.asarray(b).reshape(-1)[0])
    w6 = Wv / 6.0

    nc = _get_nc(w6, bv)

    shards = x.reshape(N_CORES, IMGS_PER_CORE * 3, P, FREE)
    in_maps = [{"x": shards[i]} for i in range(N_CORES)]
    res = run_bass_kernel_spmd(
        nc, in_maps, list(range(N_CORES)), trace=trace, tmpdir=tmpdir
    )
    out = np.stack([res.results[i]["out"] for i in range(N_CORES)], axis=0)
    out = out.reshape(32, 1, 512, 512)
    return out, res


def kernel(x, W, b):
    out, _ = _run(x, W, b, trace=False)
    return out


# revision 22
# speedup vs baseline: 1.3512x; 1.0612x over previous
"""RGB->hue + 1x1 conv (scalar scale+bias) Trainium2 Bass kernel.

Problem: x [32,3,512,512] f32 -> out [32,1,512,512] f32
  hue6 selected per argmax(r,g,b) branch:
    r max: (g-b)/delta  (mod 6)
    g max: (b-r)/delta + 2
    b max: (r-g)/delta + 4
  out = hue6 * (W/6) + b

Sharding: pure data parallel, 4 images per core on 8 cores.

Formulation ("centered hue"): let d2 = |g-b|+|b-r|+|r-g| = 2*delta
(sum of pairwise ranges of 3 scalars). Define H = hue6 if hue6<=3 else
hue6-6 (H in (-3,3]). Then
  H*delta = (g-b)        if r is max
          = (b-r) + d2/... (2*delta)  if g is max  -> dbr + d2
          = (r-g) - d2                if b is max  -> drg - d2
  (since +-2*delta == +-d2), and
  hue6-3 = add_range_wrap(H, shift=-3, bound=3, period=6)
replaces the mod-6 wrap with one custom DVE op. Final affine on ACT:
out = w6*(y+3) + bias with w6 = W/6. 1/delta comes from the ACT
Reciprocal spline as 1/(0.5*d2).
"""

import numpy as np

_EXE_CACHE: dict = {}

# Layout constants (hardcoded for x [32,3,512,512] f32, 8 cores)
N_CORES = 8
IMGS_PER_CORE = 4
P = 128              # SBUF partitions
PLANE = 512 * 512    # elements per channel plane
FREE = PLANE // P    # 2048 free-dim elements per plane
FD = 2048            # chunk free-dim size
CHUNKS = FREE // FD  # chunks per image plane


def _build(w6: float, bias: float):
    """Trace the Bass kernel with W/6 and bias baked as immediates."""
    import concourse.bacc as bacc
    import concourse.bass as bass
    import concourse.tile as tile
    from concourse import mybir

    F32 = mybir.dt.float32
    BF16 = mybir.dt.bfloat16
    U8 = mybir.dt.uint8
    U16 = mybir.dt.uint16
    Alu = mybir.AluOpType
    Act = mybir.ActivationFunctionType
    ts = bass.ts

    nc = bacc.Bacc("TRN2", target_bir_lowering=False, debug=False)

    def act_recip(out_ap, in_ap, scale=1.0):
        # Direct InstActivation emission: the bass wrapper refuses
        # Reciprocal for accuracy reasons; ~1e-4 rel here is plenty.
        ins = [
            nc.scalar.lower_ap(in_ap),
            mybir.ImmediateValue(dtype=F32, value=0.0),       # bias
            mybir.ImmediateValue(dtype=F32, value=scale),     # scale
            mybir.ImmediateValue(dtype=F32, value=0.0),       # alpha
        ]
        return nc.scalar.add_instruction(
            mybir.InstActivation(
                name=nc.get_next_instruction_name(),
                func=Act.Reciprocal,
                ins=ins,
                outs=[nc.scalar.lower_ap(out_ap)],
            )
        )

    x_t = nc.dram_tensor("x", [IMGS_PER_CORE * 3, P, FREE], F32, kind="ExternalInput")
    o_t = nc.dram_tensor("out", [IMGS_PER_CORE, P, FREE], F32, kind="ExternalOutput")

    NCHUNK = IMGS_PER_CORE * CHUNKS

    with tile.TileContext(nc, pool_alloc_mode="queue") as tc:
        with (
            tc.tile_pool(name="io", bufs=2) as io,
            tc.tile_pool(name="tmp", bufs=2) as tmp,
        ):
            state = {}

            def stage_a(ci):
                img, h = divmod(ci, CHUNKS)
                r = io.tile([P, FD], F32, tag="r")
                g = io.tile([P, FD], F32, tag="g")
                b = io.tile([P, FD], F32, tag="b")
                nc.sync.dma_start(r[:], x_t[img * 3 + 0, :, ts(h, FD)])
                nc.sync.dma_start(g[:], x_t[img * 3 + 1, :, ts(h, FD)])
                nc.sync.dma_start(b[:], x_t[img * 3 + 2, :, ts(h, FD)])

                # All elementwise on Vector/ACT only: GpSimd shares its SBUF
                # port with the DVE and slows concurrent Vector ops ~4x.
                # drg is never materialized: ndrg = dgb+dbr = -(r-g) is a
                # cheap bf16 add, and the b-branch candidate uses 2*d2-ndrg
                # (the +6*delta hue shift is absorbed by the range wrap).
                dgb = tmp.tile([P, FD], BF16, tag="dgb")
                dbr = tmp.tile([P, FD], BF16, tag="dbr")
                ndrg = tmp.tile([P, FD], BF16, tag="ndrg")
                nc.vector.tensor_sub(dgb[:], g[:], b[:])
                nc.vector.tensor_sub(dbr[:], b[:], r[:])
                nc.vector.tensor_add(ndrg[:], dgb[:], dbr[:])

                # g-branch mask on ACT: s1 = (dgb>0) u8 (Sign saturates -1
                # to 0). r-branch pre-reduce on Vector: mx = max(dbr, ndrg)
                # (r is max iff both < 0 iff mx < 0).
                s1 = tmp.tile([P, FD], U8, tag="s1")
                nc.scalar.activation(s1[:], dgb[:], Act.Sign)
                mx = tmp.tile([P, FD], BF16, tag="mx")
                nc.vector.tensor_tensor(mx[:], dbr[:], ndrg[:], op=Alu.max)
                c1 = tmp.tile([P, FD], BF16, tag="c1")
                nc.scalar.activation(c1[:], mx[:], Act.Relu, scale=-1e4)

                # d4 = 2*(|dgb|+|dbr|+|drg|) = 4*delta (abs pre-doubled, free)
                a1 = tmp.tile([P, FD], BF16, tag="a1")
                a2 = tmp.tile([P, FD], BF16, tag="a2")
                a3 = tmp.tile([P, FD], BF16, tag="a3")
                nc.scalar.activation(a1[:], dgb[:], Act.Abs, scale=2.0)
                nc.scalar.activation(a2[:], dbr[:], Act.Abs, scale=2.0)
                nc.scalar.activation(a3[:], ndrg[:], Act.Abs, scale=2.0)

                state[ci] = (dgb, dbr, ndrg, a1, a2, a3, s1, c1)

            def stage_b(ci):
                img, h = divmod(ci, CHUNKS)
                dgb, dbr, ndrg, a1, a2, a3, s1, c1 = state.pop(ci)

                d4 = tmp.tile([P, FD], BF16, tag="d4")
                nc.vector.tensor_add(d4[:], a1[:], a2[:])
                nc.vector.tensor_add(d4[:], d4[:], a3[:])

                # u = 1/delta = Recip(0.25*d4) on ACT
                u = tmp.tile([P, FD], BF16, tag="u")
                act_recip(u[:], d4[:], scale=0.25)

                # Branch candidates (H*delta; g shifted -6d, b shifted +6d —
                # both absorbed by the wrap):
                #   b-max: drg+4d = d4-ndrg (default)
                #   g-max: dbr-4d = dbr-d4, r-max: dgb
                cb = tmp.tile([P, FD], BF16, tag="cb")
                cg = tmp.tile([P, FD], BF16, tag="cg")
                nc.vector.tensor_sub(cb[:], d4[:], ndrg[:])
                nc.vector.tensor_sub(cg[:], dbr[:], d4[:])
                nc.vector.copy_predicated(cb[:], s1[:], cg[:])
                # bf16 relu mask reinterpreted as u16: nonzero iff r-max
                nc.vector.copy_predicated(cb[:], c1[:].bitcast(U16), dgb[:])

                # y = (H*delta)*(1/delta) in [-5,5]; wrap adds 6 iff y<0:
                # y2 = (y-3) + 6*[(y-3) < -3] = hue6 - 3
                nc.vector.tensor_tensor(cb[:], cb[:], u[:], op=Alu.mult)
                y2 = tmp.tile([P, FD], BF16, tag="y2")
                nc.vector.add_range_wrap(y2[:], cb[:], -3.0, 3.0, 6.0)

                # out = w6*(y2+3) + bias on ACT
                o = io.tile([P, FD], F32, tag="o")
                nc.scalar.activation(
                    o[:], y2[:], Act.Copy, bias=bias + 3.0 * w6, scale=w6
                )

                # output DMA from the (idle) PE engine's queue so it never
                # head-of-line blocks the input DMAs on sync
                nc.scalar.dma_start(o_t[img, :, ts(h, FD)], o[:])

            # software pipeline, skew 1: A(0) A(1) B(0) A(2) B(1) ...
            for ci in range(NCHUNK + 1):
                if ci < NCHUNK:
                    stage_a(ci)
                if ci >= 1:
                    stage_b(ci - 1)

    nc.compile()
    return nc


def _get_nc(w6: float, bias: float):
    key = (w6, bias, FD)
    if key not in _EXE_CACHE:
        _EXE_CACHE[key] = _build(w6, bias)
    return _EXE_CACHE[key]


def _run(x, W, b, trace=False, tmpdir=None):
    from concourse.bass_utils import run_bass_kernel_spmd

    x = np.ascontiguousarray(np.asarray(x, dtype=np.float32))
    Wv = float(np.asarray(W).reshape(-1)[0])
    bv = float(np.asarray(b).reshape(-1)[0])
    w6 = Wv / 6.0

    nc = _get_nc(w6, bv)

    shards = x.reshape(N_CORES, IMGS_PER_CORE * 3, P, FREE)
    in_maps = [{"x": shards[i]} for i in range(N_CORES)]
    res = run_bass_kernel_spmd(
        nc, in_maps, list(range(N_CORES)), trace=trace, tmpdir=tmpdir
    )
    out = np.stack([res.results[i]["out"] for i in range(N_CORES)], axis=0)
    out = out.reshape(32, 1, 512, 512)
    return out, res


def kernel(x, W, b):
    out, _ = _run(x, W, b, trace=False)
    return out


# revision 23
# speedup vs baseline: 1.3561x; 1.0036x over previous
"""RGB->hue + 1x1 conv (scalar scale+bias) Trainium2 Bass kernel.

Problem: x [32,3,512,512] f32 -> out [32,1,512,512] f32
  hue6 selected per argmax(r,g,b) branch:
    r max: (g-b)/delta  (mod 6)
    g max: (b-r)/delta + 2
    b max: (r-g)/delta + 4
  out = hue6 * (W/6) + b

Sharding: pure data parallel, 4 images per core on 8 cores.

Formulation ("centered hue"): let d2 = |g-b|+|b-r|+|r-g| = 2*delta
(sum of pairwise ranges of 3 scalars). Define H = hue6 if hue6<=3 else
hue6-6 (H in (-3,3]). Then
  H*delta = (g-b)        if r is max
          = (b-r) + d2/... (2*delta)  if g is max  -> dbr + d2
          = (r-g) - d2                if b is max  -> drg - d2
  (since +-2*delta == +-d2), and
  hue6-3 = add_range_wrap(H, shift=-3, bound=3, period=6)
replaces the mod-6 wrap with one custom DVE op. Final affine on ACT:
out = w6*(y+3) + bias with w6 = W/6. 1/delta comes from the ACT
Reciprocal spline as 1/(0.5*d2).
"""

import numpy as np

_EXE_CACHE: dict = {}

# Layout constants (hardcoded for x [32,3,512,512] f32, 8 cores)
N_CORES = 8
IMGS_PER_CORE = 4
P = 128              # SBUF partitions
PLANE = 512 * 512    # elements per channel plane
FREE = PLANE // P    # 2048 free-dim elements per plane
FD = 1024            # chunk free-dim size
CHUNKS = FREE // FD  # chunks per image plane


def _build(w6: float, bias: float):
    """Trace the Bass kernel with W/6 and bias baked as immediates."""
    import concourse.bacc as bacc
    import concourse.bass as bass
    import concourse.tile as tile
    from concourse import mybir

    F32 = mybir.dt.float32
    BF16 = mybir.dt.bfloat16
    U8 = mybir.dt.uint8
    U16 = mybir.dt.uint16
    Alu = mybir.AluOpType
    Act = mybir.ActivationFunctionType
    ts = bass.ts

    nc = bacc.Bacc("TRN2", target_bir_lowering=False, debug=False)

    def act_recip(out_ap, in_ap, scale=1.0):
        # Direct InstActivation emission: the bass wrapper refuses
        # Reciprocal for accuracy reasons; ~1e-4 rel here is plenty.
        ins = [
            nc.scalar.lower_ap(in_ap),
            mybir.ImmediateValue(dtype=F32, value=0.0),       # bias
            mybir.ImmediateValue(dtype=F32, value=scale),     # scale
            mybir.ImmediateValue(dtype=F32, value=0.0),       # alpha
        ]
        return nc.scalar.add_instruction(
            mybir.InstActivation(
                name=nc.get_next_instruction_name(),
                func=Act.Reciprocal,
                ins=ins,
                outs=[nc.scalar.lower_ap(out_ap)],
            )
        )

    x_t = nc.dram_tensor("x", [IMGS_PER_CORE * 3, P, FREE], F32, kind="ExternalInput")
    o_t = nc.dram_tensor("out", [IMGS_PER_CORE, P, FREE], F32, kind="ExternalOutput")

    NCHUNK = IMGS_PER_CORE * CHUNKS

    with tile.TileContext(nc, pool_alloc_mode="queue") as tc:
        with (
            tc.tile_pool(name="io", bufs=2) as io,
            tc.tile_pool(name="tmp", bufs=2) as tmp,
        ):
            state = {}

            def stage_a(ci):
                img, h = divmod(ci, CHUNKS)
                r = io.tile([P, FD], F32, tag="r")
                g = io.tile([P, FD], F32, tag="g")
                b = io.tile([P, FD], F32, tag="b")
                nc.sync.dma_start(r[:], x_t[img * 3 + 0, :, ts(h, FD)])
                nc.sync.dma_start(g[:], x_t[img * 3 + 1, :, ts(h, FD)])
                nc.sync.dma_start(b[:], x_t[img * 3 + 2, :, ts(h, FD)])

                # All elementwise on Vector/ACT only: GpSimd shares its SBUF
                # port with the DVE and slows concurrent Vector ops ~4x.
                # drg is never materialized: ndrg = dgb+dbr = -(r-g) is a
                # cheap bf16 add, and the b-branch candidate uses 2*d2-ndrg
                # (the +6*delta hue shift is absorbed by the range wrap).
                dgb = tmp.tile([P, FD], BF16, tag="dgb")
                dbr = tmp.tile([P, FD], BF16, tag="dbr")
                ndrg = tmp.tile([P, FD], BF16, tag="ndrg")
                nc.vector.tensor_sub(dgb[:], g[:], b[:])
                nc.vector.tensor_sub(dbr[:], b[:], r[:])
                nc.vector.tensor_add(ndrg[:], dgb[:], dbr[:])

                # g-branch mask on ACT: s1 = (dgb>0) u8 (Sign saturates -1
                # to 0). r-branch pre-reduce on Vector: mx = max(dbr, ndrg)
                # (r is max iff both < 0 iff mx < 0).
                s1 = tmp.tile([P, FD], U8, tag="s1")
                nc.scalar.activation(s1[:], dgb[:], Act.Sign)
                mx = tmp.tile([P, FD], BF16, tag="mx")
                nc.vector.tensor_tensor(mx[:], dbr[:], ndrg[:], op=Alu.max)
                c1 = tmp.tile([P, FD], BF16, tag="c1")
                nc.scalar.activation(c1[:], mx[:], Act.Relu, scale=-1e4)

                # d4 = 2*(|dgb|+|dbr|+|drg|) = 4*delta (abs pre-doubled, free)
                a1 = tmp.tile([P, FD], BF16, tag="a1")
                a2 = tmp.tile([P, FD], BF16, tag="a2")
                a3 = tmp.tile([P, FD], BF16, tag="a3")
                nc.scalar.activation(a1[:], dgb[:], Act.Abs, scale=2.0)
                nc.scalar.activation(a2[:], dbr[:], Act.Abs, scale=2.0)
                nc.scalar.activation(a3[:], ndrg[:], Act.Abs, scale=2.0)

                state[ci] = (dgb, dbr, ndrg, a1, a2, a3, s1, c1)

            def stage_b(ci):
                img, h = divmod(ci, CHUNKS)
                dgb, dbr, ndrg, a1, a2, a3, s1, c1 = state.pop(ci)

                d4 = tmp.tile([P, FD], BF16, tag="d4")
                nc.vector.tensor_add(d4[:], a1[:], a2[:])
                nc.vector.tensor_add(d4[:], d4[:], a3[:])

                # u = 1/delta = Recip(0.25*d4) on ACT
                u = tmp.tile([P, FD], BF16, tag="u")
                act_recip(u[:], d4[:], scale=0.25)

                # Branch candidates (H*delta; g shifted -6d, b shifted +6d —
                # both absorbed by the wrap):
                #   b-max: drg+4d = d4-ndrg (default)
                #   g-max: dbr-4d = dbr-d4, r-max: dgb
                cb = tmp.tile([P, FD], BF16, tag="cb")
                cg = tmp.tile([P, FD], BF16, tag="cg")
                nc.vector.tensor_sub(cb[:], d4[:], ndrg[:])
                nc.vector.tensor_sub(cg[:], dbr[:], d4[:])
                nc.vector.copy_predicated(cb[:], s1[:], cg[:])
                # bf16 relu mask reinterpreted as u16: nonzero iff r-max
                nc.vector.copy_predicated(cb[:], c1[:].bitcast(U16), dgb[:])

                # y = (H*delta)*(1/delta) in [-5,5]; wrap adds 6 iff y<0:
                # y2 = (y-3) + 6*[(y-3) < -3] = hue6 - 3
                nc.vector.tensor_tensor(cb[:], cb[:], u[:], op=Alu.mult)
                y2 = tmp.tile([P, FD], BF16, tag="y2")
                nc.vector.add_range_wrap(y2[:], cb[:], -3.0, 3.0, 6.0)

                # out = w6*(y2+3) + bias on ACT
                o = io.tile([P, FD], F32, tag="o")
                nc.scalar.activation(
                    o[:], y2[:], Act.Copy, bias=bias + 3.0 * w6, scale=w6
                )

                # output DMA from the (idle) PE engine's queue so it never
                # head-of-line blocks the input DMAs on sync
                nc.scalar.dma_start(o_t[img, :, ts(h, FD)], o[:])

            # software pipeline, skew 1: A(0) A(1) B(0) A(2) B(1) ...
            for ci in range(NCHUNK + 1):
                if ci < NCHUNK:
                    stage_a(ci)
                if ci >= 1:
                    stage_b(ci - 1)

    nc.compile()
    return nc


def _get_nc(w6: float, bias: float):
    key = (w6, bias, FD)
    if key not in _EXE_CACHE:
        _EXE_CACHE[key] = _build(w6, bias)
    return _EXE_CACHE[key]


def _run(x, W, b, trace=False, tmpdir=None):
    from concourse.bass_utils import run_bass_kernel_spmd

    x = np.ascontiguousarray(np.asarray(x, dtype=np.float32))
    Wv = float(np.asarray(W).reshape(-1)[0])
    bv = float(np.asarray(b).reshape(-1)[0])
    w6 = Wv / 6.0

    nc = _get_nc(w6, bv)

    shards = x.reshape(N_CORES, IMGS_PER_CORE * 3, P, FREE)
    in_maps = [{"x": shards[i]} for i in range(N_CORES)]
    res = run_bass_kernel_spmd(
        nc, in_maps, list(range(N_CORES)), trace=trace, tmpdir=tmpdir
    )
    out = np.stack([res.results[i]["out"] for i in range(N_CORES)], axis=0)
    out = out.reshape(32, 1, 512, 512)
    return out, res


def kernel(x, W, b):
    out, _ = _run(x, W, b, trace=False)
    return out
